# revision 1
# baseline (speedup 1.0000x reference)
"""SAGAN-style self-attention block on 8 trn2 NeuronCores.

Full inputs: x [8, 512, 64, 64], w_theta [64, 512], w_phi [64, 512],
w_g [256, 512], w_o [512, 256], gamma scalar.

Sharding: data-parallel over batch — one batch item per core. Each core runs
an identical Bass program over its own x[b]; weights are replicated.

Per-core math (C=512, n=H*W=4096, m=n/4=1024):
  theta = w_theta @ x            [64, 4096]
  phi   = pool2(w_phi @ x)       [64, 1024]
  g     = pool2(w_g @ x)         [256, 1024]
  S^T   = phi^T @ theta          [1024, 4096]   (scores, transposed layout)
  E     = exp(S^T)               (no max-subtraction needed: |S| < ~50)
  Z     = ones^T @ E             [*, 4096]      (row sums, broadcast layout)
  att   = (g @ E) / Z            [256, 4096]
  out   = (gamma*w_o) @ att + x  [512, 4096]

All matmuls run as float32r (full-rate fp32 on the PE at N>=512; tf32-like
input rounding, ~1.5e-4 rel err). The residual add uses unrounded fp32 x.
"""

import time
from contextlib import ExitStack

import numpy as np

import bass_rust
import concourse.bass as bass
import concourse.mybir as mybir
import concourse.tile as tile
from concourse.bass_utils import run_bass_kernel_spmd
from concourse.masks import make_identity

P = 128
C = 512  # channels
C8 = 64  # theta/phi channels
C2 = 256  # g channels
N = 4096  # H*W
M = 1024  # pooled spatial
NS = 8  # n-slices
SL = 512  # n-slice width
MT = 8  # m-tiles of 128
F32 = mybir.dt.float32
F32R = mybir.dt.float32r
AX = mybir.AxisListType
ALU = mybir.AluOpType
ACTF = mybir.ActivationFunctionType


def _pool_view(ap):
    """[p, 512] slice of the conv output -> 5D maxpool view [p, h2, w2, dy, dx].

    Within an n-slice of 512 = 8 image rows: local n = (2*h2+dy)*64 + 2*w2+dx.
    """
    return ap.rearrange("p (h2 dy w2 dx) -> p h2 w2 dy dx", h2=4, dy=2, w2=32, dx=2)


def emit(nc, tc, ctx):
    x_f = nc.dram_tensor("x", [C, N], F32R, kind="ExternalInput")
    wproj = nc.dram_tensor("wproj", [C, 384], F32R, kind="ExternalInput")
    wo = nc.dram_tensor("wo", [C2, C], F32R, kind="ExternalInput")
    out_d = nc.dram_tensor("out", [C, N], F32, kind="ExternalOutput")

    persist = ctx.enter_context(tc.tile_pool(name="persist", bufs=1))

    wpt = persist.tile([P, 4, 384], F32R, name="wpt")
    nc.scalar.dma_start(out=wpt, in_=wproj.ap().rearrange("(k p) o -> p k o", k=4))
    wp = [wpt[:, k, :] for k in range(4)]
    ones_f = persist.tile([P, P], F32)
    nc.vector.memset(ones_f, 1.0)
    ones = persist.tile([P, P], F32R)
    nc.vector.tensor_copy(ones, ones_f)
    ident_f = persist.tile([P, P], F32)
    make_identity(nc, ident_f)
    ident = persist.tile([P, P], F32R)
    nc.vector.tensor_copy(ident, ident_f)

    # score psum pool lives across phases 1+2 so slice-0 scores can start
    # inside phase 1
    spool = ctx.enter_context(tc.tile_pool(name="spsum", bufs=2, space="PSUM"))
    etp = ctx.enter_context(tc.tile_pool(name="et", bufs=3))
    miscp = ctx.enter_context(tc.tile_pool(name="misc", bufs=2))

    # Warm-up work for the otherwise-idle startup window (PE waits ~6us for
    # the first x data): dummy exp preloads the ACT exp table (~2.7us load
    # otherwise lands mid-phase-1 on the cast path), and a burst of matmuls
    # on constant data ramps the PE clock (HAM) before real work arrives.
    actwarm = persist.tile([P, 1], F32)
    nc.scalar.activation(actwarm, ones_f[:, 0:1], ACTF.Exp)
    for wi in range(15):
        wt_ = spool.tile([P, P], F32, name="warm", tag="s0", bufs=1)
        nc.tensor.matmul(wt_, lhsT=ones_f, rhs=ones_f, start=True, stop=True)

    # x loads: slice-major chunks so phase-1 slice 0 unblocks after ~1MB.
    # Tiles are f32r (rounded at DMA time): they feed the projection matmuls
    # directly and the residual adds read them back via bitcast — the ~1e-4
    # relative rounding on the residual is well inside the error budget.
    xf = [persist.tile([P, N], F32R, name=f"xf{cc}") for cc in range(4)]
    for q in range(NS):
        for cc in range(4):
            nc.sync.dma_start(
                out=xf[cc][:, q * SL : (q + 1) * SL],
                in_=x_f[cc * P : (cc + 1) * P, q * SL : (q + 1) * SL],
            )
    wot = []
    for k in range(2):
        t = persist.tile([P, C], F32R, name=f"wot{k}")
        nc.sync.dma_start(out=t, in_=wo[k * P : (k + 1) * P, :])
        wot.append(t)

    theta = persist.tile([C8, N], F32R)
    phi = persist.tile([P, M], F32R)  # [64:128] pooled, [0:64] copy for K rows 0-63
    g = [persist.tile([P, M], F32R, name=f"g{i}") for i in range(2)]
    gT = [persist.tile([P, C2], F32R, name=f"gT{mt}") for mt in range(MT)]

    ET = [[None] * MT for _ in range(NS)]
    FS = [[None] * (MT // 2) for _ in range(NS)]
    TH2 = [None] * NS

    def emit_th2(i):
        nsl = slice(i * SL, (i + 1) * SL)
        t = miscp.tile([P, SL], F32R, name="th2", tag="th2", bufs=2)
        nc.sync.dma_start(out=t[C8:P, :], in_=theta[:, nsl])
        TH2[i] = t

    def emit_score_pair(i, j):
        # m-tiles 2j and 2j+1 run concurrently via row tiling (separate banks)
        nsl = slice(i * SL, (i + 1) * SL)
        for half, mt in enumerate((2 * j, 2 * j + 1)):
            sp = spool.tile([P, SL], F32, name="sp", tag=f"s{half}", bufs=1)
            if half == 0:
                nc.tensor.matmul(
                    sp,
                    lhsT=phi[0:C8, mt * P : (mt + 1) * P],
                    rhs=theta[:, nsl],
                    start=True,
                    stop=True,
                )
            else:
                nc.tensor.matmul(
                    sp,
                    lhsT=phi[C8:P, mt * P : (mt + 1) * P],
                    rhs=TH2[i][C8:P, :],
                    start=True,
                    stop=True,
                    tile_position=(C8, 0),
                )
            et = etp.tile([P, SL], F32R, name="et", tag=f"et{mt}")
            nc.scalar.activation(et, sp, ACTF.Exp)
            ET[i][mt] = et

    def emit_fsums(i):
        # pair-sums on DVE, one slice ahead of the attend stage's Z matmuls
        for j in range(MT // 2):
            fsum = miscp.tile([P, SL], F32R, name="fsum", tag=f"fsum{j}", bufs=2)
            nc.vector.tensor_add(fsum, ET[i][2 * j], ET[i][2 * j + 1])
            FS[i][j] = fsum

    # ---- phase 1: projections + pooling + g transposes -----------------
    with tc.tile_pool(name="ppsum", bufs=2, space="PSUM") as pp, tc.tile_pool(
        name="tpsum", bufs=1, space="PSUM"
    ) as tp:
        for ns in range(NS):
            nsl = slice(ns * SL, (ns + 1) * SL)
            msl = slice(ns * P, (ns + 1) * P)
            xr = [xf[k][:, nsl] for k in range(4)]
            ps = [
                pp.tile(
                    [P, SL], F32, name="pp", tag=f"pp{mt}",
                    bufs=(1 if mt == 0 else 2),
                )
                for mt in range(3)
            ]
            mt_order = (1, 2, 0)
            for mt in mt_order:
                for k in range(4):
                    nc.tensor.matmul(
                        ps[mt],
                        lhsT=wp[k][:, mt * P : (mt + 1) * P],
                        rhs=xr[k],
                        start=(k == 0),
                        stop=(k == 3),
                    )
            # g pools first: with the g-first matmul order their psums are
            # ready first, and they gate this slice's transposes
            for i in range(2):
                nc.vector.tensor_reduce(
                    out=g[i][:, msl],
                    in_=_pool_view(ps[1 + i]),
                    axis=AX.XY,
                    op=ALU.max,
                )
            nc.vector.tensor_reduce(
                out=phi[C8:P, msl],
                in_=_pool_view(ps[0][C8:P, :]),
                axis=AX.XY,
                op=ALU.max,
            )
            nc.sync.dma_start(out=phi[0:C8, msl], in_=phi[C8:P, msl])
            if ns == NS - 1:
                # last slice: keep ACT free so the final score exps (which
                # gate phase-2 entry through the score-slot ring) run sooner
                nc.vector.tensor_copy(out=theta[:, nsl], in_=ps[0][0:C8, :])
            else:
                nc.scalar.copy(out=theta[:, nsl], in_=ps[0][0:C8, :])
            # transpose this slice's pooled g columns into gT[ns]
            for i in range(2):
                t = tp.tile([P, P], F32R, name="tp", tag="tp")
                nc.tensor.transpose(t, g[i][:, msl], ident)
                nc.scalar.copy(out=gT[ns][:, i * P : (i + 1) * P], in_=t)
            if ns == 0:
                emit_th2(0)
            if ns % 2 == 1:
                emit_score_pair(0, ns // 2)

    # ---- phase 2: softmax / attend / project ---------------------------
    with tc.tile_pool(name="qpsum", bufs=2, space="PSUM") as qp:
        def emit_scores(i):
            emit_th2(i)
            for j in range(MT // 2):
                emit_score_pair(i, j)

        def emit_attend(i, lo, w):
            # attend + project + residual for columns [i*SL+lo, i*SL+lo+w)
            nsl = slice(i * SL + lo, i * SL + lo + w)
            esl = slice(lo, lo + w)
            zp = qp.tile([P, w], F32, name="zp", tag="z", bufs=1)
            ap = [qp.tile([P, w], F32, name="ap", tag="a", bufs=3) for _ in range(2)]
            for mt in range(MT):
                st, sp_ = (mt == 0), (mt == MT - 1)
                if mt % 2 == 0:
                    nc.tensor.matmul(
                        zp,
                        lhsT=ones,
                        rhs=FS[i][mt // 2][:, esl],
                        start=st,
                        stop=(mt == MT - 2),
                        skip_group_check=True,
                    )
                for ct in range(2):
                    nc.tensor.matmul(
                        ap[ct],
                        lhsT=gT[mt][:, ct * P : (ct + 1) * P],
                        rhs=ET[i][mt][:, esl],
                        start=st,
                        stop=sp_,
                        skip_group_check=True,
                    )
            rinv = miscp.tile([P, w], F32, name="rinv", tag="rinv")
            nc.vector.reciprocal(rinv, zp)
            att = []
            for ct in range(2):
                t = miscp.tile([P, w], F32R, name="att", tag=f"att{ct}")
                nc.vector.tensor_mul(t, ap[ct], rinv)
                att.append(t)
            for ot in range(4):
                op_ = qp.tile([P, w], F32, name="op", tag="o")
                for ct in range(2):
                    nc.tensor.matmul(
                        op_,
                        lhsT=wot[ct][:, ot * P : (ot + 1) * P],
                        rhs=att[ct],
                        start=(ct == 0),
                        stop=(ct == 1),
                    )
                ob = miscp.tile([P, w], F32, name="ob", tag=f"ob{ot % 2}")
                nc.vector.tensor_add(ob, op_, xf[ot][:, nsl].bitcast(F32))
                nc.sync.dma_start(out=out_d[ot * P : (ot + 1) * P, nsl], in_=ob)

        emit_scores(1)
        emit_fsums(0)
        for i in range(NS):
            if i + 2 < NS:
                emit_scores(i + 2)
            if i + 1 < NS:
                emit_fsums(i + 1)
            emit_attend(i, 0, SL)


def build_nc():
    nc = bass.Bass(target_bir_lowering=False, trn_type="TRN2")
    with tile.TileContext(nc) as tc:
        with ExitStack() as ctx:
            emit(nc, tc, ctx)
    bass_rust.generate_event_semaphores(nc)
    return nc


def kernel(x, w_theta, w_phi, w_g, w_o, gamma):
    x = np.asarray(x, dtype=np.float32)
    B = x.shape[0]
    wproj = np.ascontiguousarray(
        np.concatenate(
            [np.asarray(w_theta).T, np.asarray(w_phi).T, np.asarray(w_g).T], axis=1
        ),
        dtype=np.float32,
    )
    wo_t = np.ascontiguousarray(
        (np.float32(gamma) * np.asarray(w_o)).T, dtype=np.float32
    )

    nc = build_nc()
    in_maps = []
    for b in range(B):
        xb = np.ascontiguousarray(x[b].reshape(C, N))
        in_maps.append({"x": xb, "wproj": wproj, "wo": wo_t})
    # retry: rare transient NRT_EXEC_UNIT_UNRECOVERABLE from stale device
    # state clears on re-execution
    last_err = None
    for attempt in range(3):
        try:
            res = run_bass_kernel_spmd(nc, in_maps, core_ids=list(range(B)))
            break
        except Exception as e:  # noqa: BLE001
            last_err = e
            time.sleep(2.0)
    else:
        raise last_err
    out = np.stack(
        [res.results[b]["out"].reshape(C, 64, 64) for b in range(B)]
    ).astype(np.float32)
    return out



# revision 27
# speedup vs baseline: 1.0428x; 1.0428x over previous
"""SAGAN-style self-attention block on 8 trn2 NeuronCores.

Full inputs: x [8, 512, 64, 64], w_theta [64, 512], w_phi [64, 512],
w_g [256, 512], w_o [512, 256], gamma scalar.

Sharding: data-parallel over batch — one batch item per core. Each core runs
an identical Bass program over its own x[b]; weights are replicated.

Per-core math (C=512, n=H*W=4096, m=n/4=1024):
  theta = w_theta @ x            [64, 4096]
  phi   = pool2(w_phi @ x)       [64, 1024]
  g     = pool2(w_g @ x)         [256, 1024]
  S^T   = phi^T @ theta          [1024, 4096]   (scores, transposed layout)
  E     = exp(S^T)               (no max-subtraction needed: |S| < ~50)
  Z     = ones^T @ (tree-sum E)  [*, 4096]      (row sums, broadcast layout)
  att   = (g @ E) / Z            [256, 4096]
  out   = (gamma*w_o) @ att + x  [512, 4096]

Projections and scores run as float32r (full-rate fp32 on the PE at
free>=256). The attend path (E, the exp-sum tree, g^T, att, w_o) runs in
bf16: same PE rate, half the SBUF, and 2x-rate DVE adds; the resulting
~0.2-0.4% relative error is well inside the 2e-2 budget. The residual add
uses unrounded fp32 x and fp32 psum.

Schedule: phase 1 (x-DMA paced) also runs slice-0 scores in its PE slack.
Phase 2 pipelines attend(i) | scores(i+3) | exp-tree(i+3) per iteration,
with the tree split Pool/DVE and emitted ~2 slices before its single
128-contract Z matmul. Residual outputs are written with one batched DMA
per chunk ([128, 4, w] -> four 128-row DRAM blocks).
"""

import time
from contextlib import ExitStack

import numpy as np

import bass_rust
import concourse.bass as bass
import concourse.mybir as mybir
import concourse.tile as tile
from concourse.bass_utils import run_bass_kernel_spmd
from concourse.masks import make_identity

P = 128
C = 512  # channels
C8 = 64  # theta/phi channels
C2 = 256  # g channels
N = 4096  # H*W
M = 1024  # pooled spatial
NS = 8  # n-slices
SL = 512  # n-slice width
MT = 8  # m-tiles of 128
F32 = mybir.dt.float32
F32R = mybir.dt.float32r
BF16 = mybir.dt.bfloat16
AX = mybir.AxisListType
ALU = mybir.AluOpType
ACTF = mybir.ActivationFunctionType


def _pool_view(ap):
    """[p, 512] slice of the conv output -> 5D maxpool view [p, h2, w2, dy, dx].

    Within an n-slice of 512 = 8 image rows: local n = (2*h2+dy)*64 + 2*w2+dx.
    """
    return ap.rearrange("p (h2 dy w2 dx) -> p h2 w2 dy dx", h2=4, dy=2, w2=32, dx=2)


def emit(nc, tc, ctx):
    x_f = nc.dram_tensor("x", [C, N], F32R, kind="ExternalInput")
    wproj = nc.dram_tensor("wproj", [C, 384], F32R, kind="ExternalInput")
    wo = nc.dram_tensor("wo", [C2, C], F32R, kind="ExternalInput")
    out_d = nc.dram_tensor("out", [C, N], F32, kind="ExternalOutput")
    out_v = out_d.ap().rearrange("(ot p) n -> p ot n", ot=4)

    persist = ctx.enter_context(tc.tile_pool(name="persist", bufs=1))

    # weights split per k-chunk so wp[0] lands before the x slice-0 chunks
    wpt = persist.tile([P, 4, 384], F32R, name="wpt")
    for k in range(4):
        nc.scalar.dma_start(out=wpt[:, k, :], in_=wproj.ap()[k * P : (k + 1) * P, :])
    wp = [wpt[:, k, :] for k in range(4)]
    ones_f = persist.tile([P, P], F32)
    nc.vector.memset(ones_f, 1.0)
    ones_b = persist.tile([P, P], BF16)
    nc.vector.tensor_copy(ones_b, ones_f)
    ident_f = persist.tile([P, P], F32)
    make_identity(nc, ident_f)
    ident = persist.tile([P, P], F32R)
    nc.vector.tensor_copy(ident, ident_f)

    # score psum pool lives across phases 1+2 so slice-0 can score inside
    # phase 1
    spool = ctx.enter_context(tc.tile_pool(name="spsum", bufs=2, space="PSUM"))
    etp = ctx.enter_context(tc.tile_pool(name="et", bufs=3))
    miscp = ctx.enter_context(tc.tile_pool(name="misc", bufs=2))

    # Startup: dummy exp preloads the ACT exp table (real-hw concern only);
    # the warmup matmuls start the PE p-state ramp clock early (full speed
    # ~3us after the first PE instruction) and keep the PE busy while the
    # first x/w DMAs land.
    actwarm = persist.tile([P, 1], F32)
    nc.scalar.activation(actwarm, ones_f[:, 0:1], ACTF.Exp)
    for wi in range(9):
        wt_ = spool.tile([P, P], F32, name="warm", tag=f"s{wi % 3}", bufs=1)
        nc.tensor.matmul(wt_, lhsT=ones_f, rhs=ones_f, start=True, stop=True)

    # x loads: slice-major chunks so phase-1 slice 0 unblocks after ~1MB.
    # Tiles are f32r (rounded at DMA time): they feed the projection matmuls
    # directly and the residual adds read them back via bitcast — the ~1e-4
    # relative rounding on the residual is well inside the error budget.
    xf = [persist.tile([P, N], F32R, name=f"xf{cc}") for cc in range(4)]
    for q in range(NS):
        for cc in range(4):
            nc.sync.dma_start(
                out=xf[cc][:, q * SL : (q + 1) * SL],
                in_=x_f[cc * P : (cc + 1) * P, q * SL : (q + 1) * SL],
            )
    # wot loads go after the x stream: they are not needed until the first
    # out-projection (~34us), and ahead of x they would delay phase 1
    wot = []
    for k in range(2):
        tf = persist.tile([P, C], F32R, name=f"wotf{k}")
        nc.sync.dma_start(out=tf, in_=wo[k * P : (k + 1) * P, :])
        t = persist.tile([P, C], BF16, name=f"wot{k}")
        nc.vector.tensor_copy(t, tf)
        wot.append(t)

    theta = persist.tile([C8, N], F32R)
    phi = persist.tile([C8, M], F32R)
    g = [persist.tile([P, M], F32R, name=f"g{i}") for i in range(2)]
    gT = [persist.tile([P, C2], BF16, name=f"gT{mt}") for mt in range(MT)]

    ET = [[None] * MT for _ in range(NS)]
    L1 = [None] * NS
    ZT = [None] * NS
    RINV = [None] * NS

    def emit_score(i, mt):
        nsl = slice(i * SL, (i + 1) * SL)
        sp = spool.tile([P, SL], F32, name="sp", tag=f"s{(i * MT + mt) % 3}", bufs=1)
        nc.tensor.matmul(
            sp,
            lhsT=phi[:, mt * P : (mt + 1) * P],
            rhs=theta[:, nsl],
            start=True,
            stop=True,
        )
        et = etp.tile([P, SL], BF16, name="et", tag=f"et{mt}")
        nc.scalar.activation(et, sp, ACTF.Exp)
        ET[i][mt] = et

    def emit_scores(i):
        for mt in range(MT):
            emit_score(i, mt)

    def emit_tree(i, fast=False):
        # tree-sum the 8 bf16 exp tiles so Z needs a single 128-contract
        # matmul. Levels split Pool/DVE (bf16 runs at 2x on DVE); levels 2+3
        # accumulate in place. The last DVE level (emit_l3) is emitted
        # separately so it never sits in front of an attend's reciprocal in
        # the in-order DVE queue.
        l1 = []
        for j in range(4):
            t = miscp.tile([P, SL], BF16, name="zl1", tag=f"zl1{j}", bufs=3)
            eng = nc.vector if (fast and j >= 2) or j == 3 else nc.gpsimd
            eng.tensor_add(t, ET[i][2 * j], ET[i][2 * j + 1])
            l1.append(t)
        nc.gpsimd.tensor_add(l1[0], l1[0], l1[1])
        nc.vector.tensor_add(l1[2], l1[2], l1[3])
        L1[i] = l1

    def emit_l3(i):
        l1 = L1[i]
        nc.vector.tensor_add(l1[0], l1[0], l1[2])
        ZT[i] = l1[0]

    # ---- phase 1: projections + pooling + g transposes -----------------
    with tc.tile_pool(name="ppsum", bufs=2, space="PSUM") as pp, tc.tile_pool(
        name="tpsum", bufs=1, space="PSUM"
    ) as tp:
        for ns in range(NS):
            nsl = slice(ns * SL, (ns + 1) * SL)
            msl = slice(ns * P, (ns + 1) * P)
            xr = [xf[k][:, nsl] for k in range(4)]
            ps = [
                pp.tile(
                    [P, SL], F32, name="pp", tag=f"pp{mt}",
                    bufs=(2 if mt == 2 else 1),
                )
                for mt in range(3)
            ]
            mt_order = (1, 2, 0)
            for mt in mt_order:
                for k in range(4):
                    nc.tensor.matmul(
                        ps[mt],
                        lhsT=wp[k][:, mt * P : (mt + 1) * P],
                        rhs=xr[k],
                        start=(k == 0),
                        stop=(k == 3),
                    )
            # g pools first: with the g-first matmul order their psums are
            # ready first, and they gate this slice's transposes
            for i in range(2):
                nc.vector.tensor_reduce(
                    out=g[i][:, msl],
                    in_=_pool_view(ps[1 + i]),
                    axis=AX.XY,
                    op=ALU.max,
                )
            # pooled phi written straight into partitions 0-63 (the DVE
            # access patterns cross partitions; no shift copy needed)
            nc.vector.tensor_reduce(
                out=phi[:, msl],
                in_=_pool_view(ps[0][C8:P, :]),
                axis=AX.XY,
                op=ALU.max,
            )
            if ns == NS - 1:
                # last slice: keep ACT free so the final score exps (which
                # gate phase-2 entry through the score-slot ring) run sooner
                nc.vector.tensor_copy(out=theta[:, nsl], in_=ps[0][0:C8, :])
            else:
                nc.scalar.copy(out=theta[:, nsl], in_=ps[0][0:C8, :])
            # transpose this slice's pooled g columns into gT[ns] (bf16 for
            # the attend matmuls)
            for i in range(2):
                t = tp.tile([P, P], F32R, name="tp", tag="tp")
                nc.tensor.transpose(t, g[i][:, msl], ident)
                nc.scalar.copy(out=gT[ns][:, i * P : (i + 1) * P], in_=t)
            # slice-0 scores ride in the phase-1 PE slack (x-DMA paced)
            if ns % 2 == 1:
                emit_score(0, ns - 1)
                emit_score(0, ns)
        # slice-0 tree, DVE-heavy: ready before its Z matmul a few us into
        # phase 2
        emit_tree(0, fast=True)
        emit_l3(0)

    # ---- phase 2: softmax / attend / project ---------------------------
    with tc.tile_pool(name="qpsum", bufs=2, space="PSUM") as qp:
        ATT = [None] * NS

        def emit_attend_ap(i):
            # ct-major ap accumulation; the Z matmul + reciprocal slot in
            # after the ct=0 block (ct=1 for slice 0, whose tree only
            # finishes early in phase 2) so rinv is ready for the att
            # multiplies
            ap = [qp.tile([P, SL], F32, name="ap", tag="a", bufs=2) for _ in range(2)]
            for ct in range(2):
                for mt in range(MT):
                    nc.tensor.matmul(
                        ap[ct],
                        lhsT=gT[mt][:, ct * P : (ct + 1) * P],
                        rhs=ET[i][mt],
                        start=(mt == 0),
                        stop=(mt == MT - 1),
                        skip_group_check=True,
                    )
                if ct == (1 if i == 0 else 0):
                    zp = qp.tile([P, SL], F32, name="zp", tag="z", bufs=1)
                    nc.tensor.matmul(
                        zp, lhsT=ones_b, rhs=ZT[i], start=True, stop=True,
                        skip_group_check=True,
                    )
                    rinv = miscp.tile([P, SL], F32, name="rinv", tag="rinv")
                    nc.vector.reciprocal(rinv, zp)
                    RINV[i] = rinv
            att = []
            for ct in range(2):
                t = miscp.tile([P, SL], BF16, name="att", tag=f"att{ct}", bufs=2)
                nc.vector.tensor_mul(t, ap[ct], RINV[i])
                att.append(t)
            ATT[i] = att

        def emit_outproj(i, lo, w, dma_eng=None):
            # out-projection + residual for columns [i*SL+lo, i*SL+lo+w);
            # one batched DMA writes all four 128-row DRAM blocks
            att = ATT[i]
            hsl = slice(i * SL + lo, i * SL + lo + w)
            ob = miscp.tile([P, 4, w], F32, name="ob", tag="ob", bufs=3)
            # narrow chunks: all four ot accumulations share one psum
            # allocation, so the 'o' ring never stalls the matmuls
            opq = qp.tile([P, 4, w], F32, name="op", tag="o") if w <= P else None
            for ot in range(4):
                op_ = (
                    opq[:, ot, :]
                    if opq is not None
                    else qp.tile([P, w], F32, name="op", tag="o")
                )
                for ct in range(2):
                    nc.tensor.matmul(
                        op_,
                        lhsT=wot[ct][:, ot * P : (ot + 1) * P],
                        rhs=att[ct][:, lo : lo + w],
                        start=(ct == 0),
                        stop=(ct == 1),
                        skip_group_check=True,
                    )
                nc.vector.tensor_add(
                    ob[:, ot, :], op_, xf[ot][:, hsl].bitcast(F32)
                )
            (dma_eng or nc.sync).dma_start(out=out_v[:, :, hsl], in_=ob)

        emit_scores(1)
        emit_tree(1, fast=True)
        for i in range(NS):
            emit_attend_ap(i)
            if i < NS - 2:
                chunks = [(0, SL)]
            elif i == NS - 2:
                chunks = [(0, SL // 2), (SL // 2, SL // 2)]
            else:
                chunks = [(0, SL // 2), (SL // 2, SL // 4), (3 * SL // 4, SL // 4)]
            for lo, w in chunks:
                emit_outproj(i, lo, w)
            if i + 1 < NS:
                emit_l3(i + 1)
            if i + 2 < NS:
                emit_scores(i + 2)
                emit_tree(i + 2)


def build_nc():
    nc = bass.Bass(target_bir_lowering=False, trn_type="TRN2")
    with tile.TileContext(nc) as tc:
        with ExitStack() as ctx:
            emit(nc, tc, ctx)
    bass_rust.generate_event_semaphores(nc)
    return nc


def kernel(x, w_theta, w_phi, w_g, w_o, gamma):
    x = np.asarray(x, dtype=np.float32)
    B = x.shape[0]
    wproj = np.ascontiguousarray(
        np.concatenate(
            [np.asarray(w_theta).T, np.asarray(w_phi).T, np.asarray(w_g).T], axis=1
        ),
        dtype=np.float32,
    )
    wo_t = np.ascontiguousarray(
        (np.float32(gamma) * np.asarray(w_o)).T, dtype=np.float32
    )

    nc = build_nc()
    in_maps = []
    for b in range(B):
        xb = np.ascontiguousarray(x[b].reshape(C, N))
        in_maps.append({"x": xb, "wproj": wproj, "wo": wo_t})
    # retry: rare transient NRT_EXEC_UNIT_UNRECOVERABLE from stale device
    # state clears on re-execution
    last_err = None
    for attempt in range(3):
        try:
            res = run_bass_kernel_spmd(nc, in_maps, core_ids=list(range(B)))
            break
        except Exception as e:  # noqa: BLE001
            last_err = e
            time.sleep(2.0)
    else:
        raise last_err
    out = np.stack(
        [res.results[b]["out"].reshape(C, 64, 64) for b in range(B)]
    ).astype(np.float32)
    return out


# revision 35
# speedup vs baseline: 1.0747x; 1.0306x over previous
"""SAGAN-style self-attention block on 8 trn2 NeuronCores.

Full inputs: x [8, 512, 64, 64], w_theta [64, 512], w_phi [64, 512],
w_g [256, 512], w_o [512, 256], gamma scalar.

Sharding: data-parallel over batch — one batch item per core. Each core runs
an identical Bass program over its own x[b]; weights are replicated.

Per-core math (C=512, n=H*W=4096, m=n/4=1024):
  theta = w_theta @ x            [64, 4096]
  phi   = pool2(w_phi @ x)       [64, 1024]
  g     = pool2(w_g @ x)         [256, 1024]
  S^T   = phi^T @ theta          [1024, 4096]   (scores, transposed layout)
  E     = exp(S^T)               (no max-subtraction needed: |S| < ~50)
  Z     = ones^T @ (tree-sum E)  [*, 4096]      (row sums, broadcast layout)
  att   = (g @ E) / Z            [256, 4096]
  out   = (gamma*w_o) @ att + x  [512, 4096]

Projections and scores run as float32r (full-rate fp32 on the PE at
free>=256). The attend path (E, the exp-sum tree, g^T, att, w_o) runs in
bf16: same PE rate, half the SBUF, and 2x-rate DVE adds; the resulting
~0.2-0.4% relative error is well inside the 2e-2 budget. The residual add
uses unrounded fp32 x and fp32 psum.

Schedule: phase 1 (x-DMA paced) also runs slice-0 scores in its PE slack.
Phase 2 pipelines attend(i) | scores(i+3) | exp-tree(i+3) per iteration,
with the tree split Pool/DVE and emitted ~2 slices before its single
128-contract Z matmul. Residual outputs are written with one batched DMA
per chunk ([128, 4, w] -> four 128-row DRAM blocks).
"""

import time
from contextlib import ExitStack

import numpy as np

import bass_rust
import concourse.bass as bass
import concourse.mybir as mybir
import concourse.tile as tile
from concourse.bass_utils import run_bass_kernel_spmd
from concourse.masks import make_identity

P = 128
C = 512  # channels
C8 = 64  # theta/phi channels
C2 = 256  # g channels
N = 4096  # H*W
M = 1024  # pooled spatial
NS = 8  # n-slices
SL = 512  # n-slice width
MT = 8  # m-tiles of 128
F32 = mybir.dt.float32
F32R = mybir.dt.float32r
BF16 = mybir.dt.bfloat16
AX = mybir.AxisListType
ALU = mybir.AluOpType
ACTF = mybir.ActivationFunctionType


def _pool_view(ap):
    """[p, 512] slice of the conv output -> 5D maxpool view [p, h2, w2, dy, dx].

    Within an n-slice of 512 = 8 image rows: local n = (2*h2+dy)*64 + 2*w2+dx.
    """
    return ap.rearrange("p (h2 dy w2 dx) -> p h2 w2 dy dx", h2=4, dy=2, w2=32, dx=2)


def emit(nc, tc, ctx):
    x_f = nc.dram_tensor("x", [C, N], F32R, kind="ExternalInput")
    wproj = nc.dram_tensor("wproj", [C, 384], F32R, kind="ExternalInput")
    wo = nc.dram_tensor("wo", [C2, C], F32R, kind="ExternalInput")
    out_d = nc.dram_tensor("out", [C, N], BF16, kind="ExternalOutput")
    out_v = out_d.ap().rearrange("(ot p) n -> p ot n", ot=4)

    persist = ctx.enter_context(tc.tile_pool(name="persist", bufs=1))

    wpt = persist.tile([P, 4, 384], F32R, name="wpt")
    for k in range(4):
        nc.scalar.dma_start(out=wpt[:, k, :], in_=wproj.ap()[k * P : (k + 1) * P, :])
    wp = [wpt[:, k, :] for k in range(4)]
    ones_f = persist.tile([P, P], F32)
    nc.vector.memset(ones_f, 1.0)
    ones_b = persist.tile([P, P], BF16)
    nc.vector.tensor_copy(ones_b, ones_f)
    ident_f = persist.tile([P, P], F32)
    make_identity(nc, ident_f)
    ident = persist.tile([P, P], F32R)
    nc.vector.tensor_copy(ident, ident_f)

    # score psum pool lives across phases 1+2 so slice-0 can score inside
    # phase 1
    spool = ctx.enter_context(tc.tile_pool(name="spsum", bufs=2, space="PSUM"))
    etp = ctx.enter_context(tc.tile_pool(name="et", bufs=3))
    miscp = ctx.enter_context(tc.tile_pool(name="misc", bufs=2))

    # Startup: dummy exp preloads the ACT exp table (real-hw concern only);
    # the warmup matmuls start the PE p-state ramp clock early (full speed
    # ~3us after the first PE instruction) and keep the PE busy while the
    # first x/w DMAs land.
    actwarm = persist.tile([P, 1], F32)
    nc.scalar.activation(actwarm, ones_f[:, 0:1], ACTF.Exp)
    for wi in range(9):
        wt_ = spool.tile([P, P], F32, name="warm", tag=f"s{wi % 3}", bufs=1)
        nc.tensor.matmul(wt_, lhsT=ones_f, rhs=ones_f, start=True, stop=True)

    # x loads: slice-major chunks so phase-1 slice 0 unblocks after ~1MB.
    # Tiles are f32r (rounded at DMA time): they feed the projection matmuls
    # directly and the residual adds read them back via bitcast — the ~1e-4
    # relative rounding on the residual is well inside the error budget.
    xfa = persist.tile([P, 4, N], F32R, name="xfa")
    for q in range(NS):
        for cc in range(4):
            nc.sync.dma_start(
                out=xfa[:, cc, q * SL : (q + 1) * SL],
                in_=x_f[cc * P : (cc + 1) * P, q * SL : (q + 1) * SL],
            )
    # wot loads go after the x stream: they are not needed until the first
    # out-projection (~34us), and ahead of x they would delay phase 1
    wot = []
    for k in range(2):
        tf = persist.tile([P, C], F32R, name=f"wotf{k}")
        nc.sync.dma_start(out=tf, in_=wo[k * P : (k + 1) * P, :])
        t = persist.tile([P, C], BF16, name=f"wot{k}")
        nc.gpsimd.tensor_copy(t, tf)
        wot.append(t)

    theta = persist.tile([C8, N], F32R)
    phi = persist.tile([C8, M], F32R)
    g = [persist.tile([P, M], F32R, name=f"g{i}") for i in range(2)]
    gT = [persist.tile([P, C2], BF16, name=f"gT{mt}") for mt in range(MT)]

    ET = [[None] * MT for _ in range(NS)]
    L1 = [None] * NS
    ZT = [None] * NS
    RINV = [None] * NS

    def emit_score(i, mt):
        nsl = slice(i * SL, (i + 1) * SL)
        sp = spool.tile([P, SL], F32, name="sp", tag=f"s{(i * MT + mt) % 3}", bufs=1)
        nc.tensor.matmul(
            sp,
            lhsT=phi[:, mt * P : (mt + 1) * P],
            rhs=theta[:, nsl],
            start=True,
            stop=True,
        )
        et = etp.tile([P, SL], BF16, name="et", tag=f"et{mt}")
        nc.scalar.activation(et, sp, ACTF.Exp)
        ET[i][mt] = et

    def emit_scores(i):
        for mt in range(MT):
            emit_score(i, mt)

    def emit_tree(i, fast=False):
        # tree-sum the 8 bf16 exp tiles so Z needs a single 128-contract
        # matmul. Levels split Pool/DVE (bf16 runs at 2x on DVE); levels 2+3
        # accumulate in place. The last DVE level (emit_l3) is emitted
        # separately so it never sits in front of an attend's reciprocal in
        # the in-order DVE queue.
        l1 = []
        for j in range(4):
            t = miscp.tile([P, SL], BF16, name="zl1", tag=f"zl1{j}", bufs=3)
            eng = nc.vector if (fast and j >= 2) or j == 3 else nc.gpsimd
            eng.tensor_add(t, ET[i][2 * j], ET[i][2 * j + 1])
            l1.append(t)
        nc.gpsimd.tensor_add(l1[0], l1[0], l1[1])
        nc.vector.tensor_add(l1[2], l1[2], l1[3])
        L1[i] = l1

    def emit_l3(i):
        l1 = L1[i]
        nc.vector.tensor_add(l1[0], l1[0], l1[2])
        ZT[i] = l1[0]

    # ---- phase 1: projections + pooling + g transposes -----------------
    with tc.tile_pool(name="ppsum", bufs=2, space="PSUM") as pp, tc.tile_pool(
        name="tpsum", bufs=1, space="PSUM"
    ) as tp:
        for ns in range(NS):
            nsl = slice(ns * SL, (ns + 1) * SL)
            msl = slice(ns * P, (ns + 1) * P)
            xr = [xfa[:, k, nsl] for k in range(4)]
            ps = [
                pp.tile(
                    [P, SL], F32, name="pp", tag=f"pp{mt}",
                    bufs=(2 if mt == 2 else 1),
                )
                for mt in range(3)
            ]
            # last slice: k-major so only three matmuls wait on the final
            # x chunk (no successor slice to stall on the pool reads)
            if ns == NS - 1:
                order = [(mt, k) for k in range(4) for mt in (1, 2, 0)]
            else:
                order = [(mt, k) for mt in (1, 2, 0) for k in range(4)]
            for mt, k in order:
                nc.tensor.matmul(
                    ps[mt],
                    lhsT=wp[k][:, mt * P : (mt + 1) * P],
                    rhs=xr[k],
                    start=(k == 0),
                    stop=(k == 3),
                    skip_group_check=True,
                )

            # g pools first: with the g-first matmul order their psums are
            # ready first, and they gate this slice's transposes
            for i in range(2):
                nc.vector.tensor_reduce(
                    out=g[i][:, msl],
                    in_=_pool_view(ps[1 + i]),
                    axis=AX.XY,
                    op=ALU.max,
                )
            # pooled phi written straight into partitions 0-63 (the DVE
            # access patterns cross partitions; no shift copy needed)
            nc.vector.tensor_reduce(
                out=phi[:, msl],
                in_=_pool_view(ps[0][C8:P, :]),
                axis=AX.XY,
                op=ALU.max,
            )
            if ns == NS - 1:
                # last slice: keep ACT free so the final score exps (which
                # gate phase-2 entry through the score-slot ring) run sooner
                nc.vector.tensor_copy(out=theta[:, nsl], in_=ps[0][0:C8, :])
            else:
                nc.scalar.copy(out=theta[:, nsl], in_=ps[0][0:C8, :])
            # transpose this slice's pooled g columns into gT[ns] (bf16 for
            # the attend matmuls)
            for i in range(2):
                t = tp.tile([P, P], F32R, name="tp", tag="tp")
                nc.tensor.transpose(t, g[i][:, msl], ident)
                nc.scalar.copy(out=gT[ns][:, i * P : (i + 1) * P], in_=t)
            # slice-0 scores ride in the phase-1 PE slack (x-DMA paced)
            if ns == 1:
                emit_score(0, 0)
            if ns >= 1:
                emit_score(0, ns)
        # slice-0 tree, DVE-heavy: ready before its Z matmul a few us into
        # phase 2
        emit_tree(0, fast=True)
        emit_l3(0)

    # ---- phase 2: softmax / attend / project ---------------------------
    with tc.tile_pool(name="qpsum", bufs=2, space="PSUM") as qp:
        ATT = [None] * NS

        def emit_attend_ap(i):
            # ct-major ap accumulation; the Z matmul + reciprocal slot in
            # after the ct=0 block (ct=1 for slice 0, whose tree only
            # finishes early in phase 2) so rinv is ready for the att
            # multiplies
            ap = [qp.tile([P, SL], F32, name="ap", tag="a", bufs=2) for _ in range(2)]
            for ct in range(2):
                for mt in range(MT):
                    nc.tensor.matmul(
                        ap[ct],
                        lhsT=gT[mt][:, ct * P : (ct + 1) * P],
                        rhs=ET[i][mt],
                        start=(mt == 0),
                        stop=(mt == MT - 1),
                        skip_group_check=True,
                    )
                if ct == (1 if i == 0 else 0):
                    zp = qp.tile([P, SL], F32, name="zp", tag="z", bufs=1)
                    nc.tensor.matmul(
                        zp, lhsT=ones_b, rhs=ZT[i], start=True, stop=True,
                        skip_group_check=True,
                    )
                    rinv = miscp.tile([P, SL], F32, name="rinv", tag="rinv")
                    nc.vector.reciprocal(rinv, zp)
                    RINV[i] = rinv
            att = []
            for ct in range(2):
                t = miscp.tile([P, SL], BF16, name="att", tag=f"att{ct}", bufs=2)
                nc.vector.tensor_mul(t, ap[ct], RINV[i])
                att.append(t)
            ATT[i] = att

        def emit_outproj(i, lo, w, dma_eng=None, tags=None):
            # out-projection + residual for columns [i*SL+lo, i*SL+lo+w);
            # one batched DMA writes all four 128-row DRAM blocks. Narrow
            # chunks group the ot accumulations into shared psum allocations
            # (fewer 'o' ring waits) and do one batched residual add per
            # group instead of four.
            att = ATT[i]
            hsl = slice(i * SL + lo, i * SL + lo + w)
            ob = miscp.tile([P, 4, w], BF16, name="ob", tag="ob", bufs=4)
            if w <= P:
                groups = [(0, 4)]
            elif w <= 2 * P:
                groups = [(0, 2), (2, 2)]
            else:
                groups = [(ot, 1) for ot in range(4)]
            for gi, (base, cnt) in enumerate(groups):
                gtag = tags[gi] if tags else "o"
                opg = qp.tile([P, cnt, w], F32, name="op", tag=gtag)
                for d in range(cnt):
                    for ct in range(2):
                        nc.tensor.matmul(
                            opg[:, d, :],
                            lhsT=wot[ct][:, (base + d) * P : (base + d + 1) * P],
                            rhs=att[ct][:, lo : lo + w],
                            start=(ct == 0),
                            stop=(ct == 1),
                            skip_group_check=True,
                        )
                nc.vector.tensor_add(
                    ob[:, base : base + cnt, :],
                    opg,
                    xfa[:, base : base + cnt, hsl].bitcast(F32),
                )
            (dma_eng or nc.sync).dma_start(out=out_v[:, :, hsl], in_=ob)

        emit_scores(1)
        emit_tree(1, fast=True)
        for i in range(NS):
            emit_attend_ap(i)
            # the next slice's final tree add goes ahead of this slice's
            # residual adds in the DVE queue so its Z matmul never stalls
            if i + 1 < NS:
                emit_l3(i + 1)
            if i < NS - 1:
                chunks = [(0, SL)]
            else:
                chunks = [(0, SL // 2), (SL // 2, SL // 2)]
            ctags = [None, None]
            if i == NS - 1:
                # the attmuls have released the 'a' psum ring by now;
                # alternating 'o'/'a' removes every endgame ring wait
                ctags = [("o", "a"), ("o", "a")]
            for ci, (lo, w) in enumerate(chunks):
                emit_outproj(
                    i, lo, w,
                    dma_eng=nc.scalar if ci % 2 else None,
                    tags=ctags[ci],
                )
            if i + 2 < NS:
                emit_scores(i + 2)
                emit_tree(i + 2)


def build_nc():
    nc = bass.Bass(target_bir_lowering=False, trn_type="TRN2")
    with tile.TileContext(nc) as tc:
        with ExitStack() as ctx:
            emit(nc, tc, ctx)
    bass_rust.generate_event_semaphores(nc)
    return nc


def kernel(x, w_theta, w_phi, w_g, w_o, gamma):
    x = np.asarray(x, dtype=np.float32)
    B = x.shape[0]
    wproj = np.ascontiguousarray(
        np.concatenate(
            [np.asarray(w_theta).T, np.asarray(w_phi).T, np.asarray(w_g).T], axis=1
        ),
        dtype=np.float32,
    )
    wo_t = np.ascontiguousarray(
        (np.float32(gamma) * np.asarray(w_o)).T, dtype=np.float32
    )

    nc = build_nc()
    in_maps = []
    for b in range(B):
        xb = np.ascontiguousarray(x[b].reshape(C, N))
        in_maps.append({"x": xb, "wproj": wproj, "wo": wo_t})
    # retry: rare transient NRT_EXEC_UNIT_UNRECOVERABLE from stale device
    # state clears on re-execution
    last_err = None
    for attempt in range(3):
        try:
            res = run_bass_kernel_spmd(nc, in_maps, core_ids=list(range(B)))
            break
        except Exception as e:  # noqa: BLE001
            last_err = e
            time.sleep(2.0)
    else:
        raise last_err
    out = np.stack(
        [res.results[b]["out"].reshape(C, 64, 64) for b in range(B)]
    ).astype(np.float32)
    return out


# revision 61
# speedup vs baseline: 1.0857x; 1.0103x over previous
"""SAGAN-style self-attention block on 8 trn2 NeuronCores.

Full inputs: x [8, 512, 64, 64], w_theta [64, 512], w_phi [64, 512],
w_g [256, 512], w_o [512, 256], gamma scalar.

Sharding: data-parallel over batch — one batch item per core. Each core runs
an identical Bass program over its own x[b]; weights are replicated.

Per-core math (C=512, n=H*W=4096, m=n/4=1024):
  theta = w_theta @ x            [64, 4096]
  phi   = pool2(w_phi @ x)       [64, 1024]
  g     = pool2(w_g @ x)         [256, 1024]
  S^T   = phi^T @ theta          [1024, 4096]   (scores, transposed layout)
  E     = exp(S^T)               (no max-subtraction needed: |S| < ~50)
  Z     = ones^T @ (tree-sum E)  [*, 4096]      (row sums, broadcast layout)
  att   = (g @ E) / Z            [256, 4096]
  out   = (gamma*w_o) @ att + x  [512, 4096]

Projections and scores run as float32r (full-rate fp32 on the PE at
free>=256). The attend path (E, the exp-sum tree, g^T, att, w_o) runs in
bf16: same PE rate, half the SBUF, and 2x-rate DVE adds; the resulting
~0.2-0.4% relative error is well inside the 2e-2 budget. The residual add
uses unrounded fp32 x and fp32 psum.

Schedule: phase 1 (x-DMA paced) also runs slice-0 scores in its PE slack.
Phase 2 pipelines attend(i) | scores(i+3) | exp-tree(i+3) per iteration,
with the tree split Pool/DVE and emitted ~2 slices before its single
128-contract Z matmul. Residual outputs are written with one batched DMA
per chunk ([128, 4, w] -> four 128-row DRAM blocks).
"""

import time
from contextlib import ExitStack

import numpy as np

import bass_rust
import concourse.bass as bass
import concourse.mybir as mybir
import concourse.tile as tile
from concourse.bass_utils import run_bass_kernel_spmd
from concourse.masks import make_identity

P = 128
C = 512  # channels
C8 = 64  # theta/phi channels
C2 = 256  # g channels
N = 4096  # H*W
M = 1024  # pooled spatial
NS = 8  # n-slices
SL = 512  # n-slice width
MT = 8  # m-tiles of 128
F32 = mybir.dt.float32
F32R = mybir.dt.float32r
BF16 = mybir.dt.bfloat16
AX = mybir.AxisListType
ALU = mybir.AluOpType
ACTF = mybir.ActivationFunctionType


def _pool_view(ap):
    """[p, 512] slice of the conv output -> 5D maxpool view [p, h2, w2, dy, dx].

    Within an n-slice of 512 = 8 image rows: local n = (2*h2+dy)*64 + 2*w2+dx.
    """
    return ap.rearrange("p (h2 dy w2 dx) -> p h2 w2 dy dx", h2=4, dy=2, w2=32, dx=2)


def emit(nc, tc, ctx):
    x_f = nc.dram_tensor("x", [C, N], F32R, kind="ExternalInput")
    wproj = nc.dram_tensor("wproj", [C, 384], F32R, kind="ExternalInput")
    wo = nc.dram_tensor("wo", [C2, C], F32R, kind="ExternalInput")
    out_d = nc.dram_tensor("out", [C, N], BF16, kind="ExternalOutput")
    out_v = out_d.ap().rearrange("(ot p) n -> p ot n", ot=4)

    persist = ctx.enter_context(tc.tile_pool(name="persist", bufs=1))

    wpt = persist.tile([P, 4, 384], F32R, name="wpt")
    for k in range(4):
        nc.scalar.dma_start(out=wpt[:, k, :], in_=wproj.ap()[k * P : (k + 1) * P, :])
    wp = [wpt[:, k, :] for k in range(4)]
    ones_f = persist.tile([P, P], F32)
    nc.vector.memset(ones_f, 1.0)
    ones_b = persist.tile([P, P], BF16)
    nc.vector.tensor_copy(ones_b, ones_f)
    ident_f = persist.tile([P, P], F32)
    make_identity(nc, ident_f)
    ident = persist.tile([P, P], F32R)
    nc.vector.tensor_copy(ident, ident_f)

    # score psum pool lives across phases 1+2 so slice-0 can score inside
    # phase 1
    spool = ctx.enter_context(tc.tile_pool(name="spsum", bufs=2, space="PSUM"))
    etp = ctx.enter_context(tc.tile_pool(name="et", bufs=3))
    miscp = ctx.enter_context(tc.tile_pool(name="misc", bufs=2))

    # Startup: dummy exp preloads the ACT exp table (real-hw concern only);
    # the warmup matmuls start the PE p-state ramp clock early (full speed
    # ~3us after the first PE instruction) and keep the PE busy while the
    # first x/w DMAs land.
    actwarm = persist.tile([P, 1], F32)
    nc.scalar.activation(actwarm, ones_f[:, 0:1], ACTF.Exp)
    for wi in range(11):
        wt_ = spool.tile([P, P], F32, name="warm", tag=f"s{wi % 4}", bufs=1)
        nc.tensor.matmul(wt_, lhsT=ones_f, rhs=ones_f, start=True, stop=True)

    # x loads: slice-major chunks so phase-1 slice 0 unblocks after ~1MB.
    # Tiles are f32r (rounded at DMA time): they feed the projection matmuls
    # directly and the residual adds read them back via bitcast — the ~1e-4
    # relative rounding on the residual is well inside the error budget.
    xfa = persist.tile([P, 4, N], F32R, name="xfa")
    for q in range(NS):
        for cc in range(4):
            nc.sync.dma_start(
                out=xfa[:, cc, q * SL : (q + 1) * SL],
                in_=x_f[cc * P : (cc + 1) * P, q * SL : (q + 1) * SL],
            )
    # wot loads go after the x stream: they are not needed until the first
    # out-projection (~34us), and ahead of x they would delay phase 1
    wot = []
    for k in range(2):
        tf = persist.tile([P, C], F32R, name=f"wotf{k}")
        nc.sync.dma_start(out=tf, in_=wo[k * P : (k + 1) * P, :])
        t = persist.tile([P, C], BF16, name=f"wot{k}")
        nc.gpsimd.tensor_copy(t, tf)
        wot.append(t)

    theta = persist.tile([C8, N], F32R)
    phi = persist.tile([C8, M], F32R)
    g = [persist.tile([P, M], F32R, name=f"g{i}") for i in range(2)]
    gT = [persist.tile([P, C2], BF16, name=f"gT{mt}") for mt in range(MT)]

    ET = [[None] * MT for _ in range(NS)]
    L1 = [None] * NS
    ZT = [None] * NS
    RINV = [None] * NS

    def emit_score(i, mt):
        nsl = slice(i * SL, (i + 1) * SL)
        sp = spool.tile([P, SL], F32, name="sp", tag=f"s{(i * MT + mt) % 4}", bufs=1)
        nc.tensor.matmul(
            sp,
            lhsT=phi[:, mt * P : (mt + 1) * P],
            rhs=theta[:, nsl],
            start=True,
            stop=True,
        )
        et = etp.tile([P, SL], BF16, name="et", tag=f"et{mt}")
        nc.scalar.activation(et, sp, ACTF.Exp)
        ET[i][mt] = et

    def emit_scores(i):
        for mt in range(MT):
            emit_score(i, mt)

    def emit_tree(i, fast=False):
        # tree-sum the 8 bf16 exp tiles so Z needs a single 128-contract
        # matmul. Levels split Pool/DVE (bf16 runs at 2x on DVE); levels 2+3
        # accumulate in place. The last DVE level (emit_l3) is emitted
        # separately so it never sits in front of an attend's reciprocal in
        # the in-order DVE queue.
        l1 = []
        for j in range(4):
            t = miscp.tile([P, SL], BF16, name="zl1", tag=f"zl1{j}", bufs=3)
            eng = nc.vector if (fast and j >= 2) or j == 3 else nc.gpsimd
            eng.tensor_add(t, ET[i][2 * j], ET[i][2 * j + 1])
            l1.append(t)
        nc.gpsimd.tensor_add(l1[0], l1[0], l1[1])
        nc.vector.tensor_add(l1[2], l1[2], l1[3])
        L1[i] = l1

    def emit_l3(i):
        l1 = L1[i]
        nc.vector.tensor_add(l1[0], l1[0], l1[2])
        ZT[i] = l1[0]

    # ---- phase 1: projections + pooling + g transposes -----------------
    with tc.tile_pool(name="ppsum", bufs=2, space="PSUM") as pp, tc.tile_pool(
        name="tpsum", bufs=1, space="PSUM"
    ) as tp:
        for ns in range(NS):
            nsl = slice(ns * SL, (ns + 1) * SL)
            msl = slice(ns * P, (ns + 1) * P)
            xr = [xfa[:, k, nsl] for k in range(4)]
            ps = [
                pp.tile(
                    [P, SL], F32, name="pp", tag=f"pp{mt}", bufs=1,
                )
                for mt in range(3)
            ]
            # last slice: k-major so only three matmuls wait on the final
            # x chunk (no successor slice to stall on the pool reads)
            if ns == NS - 1:
                order = [(mt, k) for k in range(4) for mt in (0, 1, 2)]
            else:
                order = [(mt, k) for mt in (1, 2, 0) for k in range(4)]
            for mt, k in order:
                nc.tensor.matmul(
                    ps[mt],
                    lhsT=wp[k][:, mt * P : (mt + 1) * P],
                    rhs=xr[k],
                    start=(k == 0),
                    stop=(k == 3),
                    skip_group_check=True,
                )

            # pooled phi written straight into partitions 0-63 (the DVE
            # access patterns cross partitions; no shift copy needed).
            # On the last slice phi + theta go first: they gate the final
            # scores and with them the whole phase-2 entry.
            def _pool_phi():
                nc.vector.tensor_reduce(
                    out=phi[:, msl],
                    in_=_pool_view(ps[0][C8:P, :]),
                    axis=AX.XY,
                    op=ALU.max,
                )

            def _pool_g():
                for i in range(2):
                    nc.vector.tensor_reduce(
                        out=g[i][:, msl],
                        in_=_pool_view(ps[1 + i]),
                        axis=AX.XY,
                        op=ALU.max,
                    )

            if ns == NS - 1:
                _pool_phi()
                # keep ACT free so the final score exps (which gate phase-2
                # entry through the score-slot ring) run sooner
                nc.vector.tensor_copy(out=theta[:, nsl], in_=ps[0][0:C8, :])
                emit_score(0, ns)
                _pool_g()
            else:
                _pool_g()
                _pool_phi()
                nc.scalar.copy(out=theta[:, nsl], in_=ps[0][0:C8, :])
            # transpose this slice's pooled g columns into gT[ns] (bf16 for
            # the attend matmuls)
            for i in range(2):
                t = tp.tile([P, P], F32R, name="tp", tag="tp")
                nc.tensor.transpose(t, g[i][:, msl], ident)
                nc.scalar.copy(out=gT[ns][:, i * P : (i + 1) * P], in_=t)
            # slice-0/1 scores ride in the phase-1 PE slack (x-DMA paced)
            if ns == 1:
                emit_score(0, 0)
            if 1 <= ns < NS - 1:
                emit_score(0, ns)
            if 2 <= ns < NS - 1:
                emit_score(1, ns - 2)
        # slice-0 tree, DVE-heavy: ready before its Z matmul a few us into
        # phase 2
        emit_tree(0, fast=True)
        emit_l3(0)

    # ---- phase 2: softmax / attend / project ---------------------------
    with tc.tile_pool(name="qpsum", bufs=2, space="PSUM") as qp:
        ATT = [None] * NS

        def emit_attend_ap(i):
            # ct-major ap accumulation; the Z matmul + reciprocal slot in
            # after the ct=0 block (ct=1 for slice 0, whose tree only
            # finishes early in phase 2) so rinv is ready for the att
            # multiplies
            ap = [qp.tile([P, SL], F32, name="ap", tag="a", bufs=2) for _ in range(2)]
            for ct in range(2):
                for mt in range(MT):
                    nc.tensor.matmul(
                        ap[ct],
                        lhsT=gT[mt][:, ct * P : (ct + 1) * P],
                        rhs=ET[i][mt],
                        start=(mt == 0),
                        stop=(mt == MT - 1),
                        skip_group_check=True,
                    )
                if ct == (1 if i == 0 else 0):
                    # zp borrows an 'o' ring slot (free mid-attend), leaving
                    # a psum bank for a fourth score tag
                    zp = qp.tile([P, SL], F32, name="zp", tag="o")
                    nc.tensor.matmul(
                        zp, lhsT=ones_b, rhs=ZT[i], start=True, stop=True,
                        skip_group_check=True,
                    )
                    rinv = miscp.tile([P, SL], F32, name="rinv", tag="rinv")
                    nc.vector.reciprocal(rinv, zp)
                    RINV[i] = rinv
            att = []
            for ct in range(2):
                t = miscp.tile([P, SL], BF16, name="att", tag=f"att{ct}", bufs=2)
                nc.vector.tensor_mul(t, ap[ct], RINV[i])
                att.append(t)
            ATT[i] = att

        def emit_outproj(i, lo, w, dma_eng=None, tags=None, fine_obs=False,
                         pool_obs=False):
            # out-projection + residual for columns [i*SL+lo, i*SL+lo+w);
            # one batched DMA writes all four 128-row DRAM blocks. Narrow
            # chunks group the ot accumulations into shared psum allocations
            # (fewer 'o' ring waits) and do one batched residual add per
            # group instead of four.
            att = ATT[i]
            hsl = slice(i * SL + lo, i * SL + lo + w)
            ob = miscp.tile([P, 4, w], BF16, name="ob", tag="ob", bufs=4)
            if w <= P:
                groups = [(0, 4)]
            elif w <= 2 * P:
                groups = [(0, 2), (2, 2)]
            else:
                groups = [(ot, 1) for ot in range(4)]
            for gi, (base, cnt) in enumerate(groups):
                gtag = tags[gi] if tags else "o"
                opg = qp.tile([P, cnt, w], F32, name="op", tag=gtag)
                for d in range(cnt):
                    for ct in range(2):
                        nc.tensor.matmul(
                            opg[:, d, :],
                            lhsT=wot[ct][:, (base + d) * P : (base + d + 1) * P],
                            rhs=att[ct][:, lo : lo + w],
                            start=(ct == 0),
                            stop=(ct == 1),
                            skip_group_check=True,
                        )
                if pool_obs and base >= 2:
                    # second half of the residual adds on Pool: frees the
                    # DVE for the next slice's reciprocal/att multiplies
                    nc.gpsimd.tensor_add(
                        ob[:, base : base + cnt, :],
                        opg,
                        xfa[:, base : base + cnt, hsl].bitcast(F32),
                    )
                elif fine_obs:
                    # per-ot adds: the last one trails the final matmul by
                    # only ~0.4us, so the closing DMA starts sooner
                    for d in range(cnt):
                        nc.vector.tensor_add(
                            ob[:, base + d, :],
                            opg[:, d, :],
                            xfa[:, base + d, hsl].bitcast(F32),
                        )
                else:
                    nc.vector.tensor_add(
                        ob[:, base : base + cnt, :],
                        opg,
                        xfa[:, base : base + cnt, hsl].bitcast(F32),
                    )
            (dma_eng or nc.sync).dma_start(out=out_v[:, :, hsl], in_=ob)

        for mt in range(NS - 3, MT):
            emit_score(1, mt)
        emit_tree(1, fast=True)
        for i in range(NS):
            emit_attend_ap(i)
            # the next slice's final tree add goes ahead of this slice's
            # residual adds in the DVE queue so its Z matmul never stalls
            if i + 1 < NS:
                emit_l3(i + 1)
            if i < NS - 1:
                chunks = [(0, SL)]
            else:
                chunks = [(0, SL // 2), (SL // 2, SL // 2)]
            ctags = [None, None]
            if i == NS - 1:
                # the attmuls have released the 'a' psum ring by now;
                # alternating 'o'/'a' removes every endgame ring wait
                ctags = [("o", "a"), ("o", "a")]
            for ci, (lo, w) in enumerate(chunks):
                emit_outproj(
                    i, lo, w,
                    dma_eng=nc.scalar if ci % 2 else None,
                    tags=ctags[ci],
                    fine_obs=False,
                    pool_obs=False,
                )
            if i + 2 < NS:
                emit_scores(i + 2)
                emit_tree(i + 2)


def build_nc():
    nc = bass.Bass(target_bir_lowering=False, trn_type="TRN2")
    with tile.TileContext(nc) as tc:
        with ExitStack() as ctx:
            emit(nc, tc, ctx)
    bass_rust.generate_event_semaphores(nc)
    return nc


def kernel(x, w_theta, w_phi, w_g, w_o, gamma):
    x = np.asarray(x, dtype=np.float32)
    B = x.shape[0]
    wproj = np.ascontiguousarray(
        np.concatenate(
            [np.asarray(w_theta).T, np.asarray(w_phi).T, np.asarray(w_g).T], axis=1
        ),
        dtype=np.float32,
    )
    wo_t = np.ascontiguousarray(
        (np.float32(gamma) * np.asarray(w_o)).T, dtype=np.float32
    )

    nc = build_nc()
    in_maps = []
    for b in range(B):
        xb = np.ascontiguousarray(x[b].reshape(C, N))
        in_maps.append({"x": xb, "wproj": wproj, "wo": wo_t})
    # retry: rare transient NRT_EXEC_UNIT_UNRECOVERABLE from stale device
    # state clears on re-execution
    last_err = None
    for attempt in range(3):
        try:
            res = run_bass_kernel_spmd(nc, in_maps, core_ids=list(range(B)))
            break
        except Exception as e:  # noqa: BLE001
            last_err = e
            time.sleep(2.0)
    else:
        raise last_err
    out = np.stack(
        [res.results[b]["out"].reshape(C, 64, 64) for b in range(B)]
    ).astype(np.float32)
    return out
